# revision 6
# baseline (speedup 1.0000x reference)
"""AttentionBlock Trainium2 kernel.

Reference computation (per batch b):
    xf = x[b].reshape(N, C);  N = 64*64 = 4096, C = 256, d = C//8 = 32
    q = xf @ Wq + bq; k = xf @ Wk + bk; v = xf @ Wv + bv
    out = softmax(q @ k.T) @ v
    y = gamma * out + xf

Sharding: 8 cores = 4 batches x 2 halves of the query rows. Each core
computes k/v for its full batch and attention for its 2048 query rows.

Per-core kernel design (v2):
  - Host passes xT (x[b] transposed, own query half rolled to the front) so
    all projection matmuls contract over channels on the partition dim.
  - q/k are projected with 4x-replicated weights so the d=32 score
    contraction runs as FOUR concurrent tile_position row-strip matmuls
    (K=32 each) - ~3x effective PE throughput on the scores.
  - Scores are computed TRANSPOSED (scoresT[m, n] = k[m].q[n]) so the exp'd
    weights feed attn@v as the stationary operand with no transposes.
  - exp runs on ScalarE (the only exp engine) in [128, 4, 512] blocks
    straight out of PSUM, emitting BF16: halves LDWEIGHTS traffic in the
    attn@v stream (FWL reads bf16 weights 2x faster) at no measurable
    accuracy cost (validated: rel err 3.6e-4 vs 2e-2 budget).
  - v (bf16, +ones column for the softmax denominator) is projected into
    vaug; attn@v accumulates all 32 key tiles into a 4-bank PSUM acc.
  - The head is fully pipelined: DMA descriptors spread over 4 engine
    queues, PE warmup gated only on a small const chunk, projections
    interleaved with x-chunk arrival region by region, and slice 0's
    scores+exp interleaved into the projection phase so the ScalarE exp
    stream (the end-to-end bottleneck, ~64us/core) starts at ~4us.
"""

import numpy as np

CH = 256
DQK = 32
N = 4096  # H*W
NQ = 2048  # query rows per core
B = 4
N_CORES = 8
CH2 = CH + 2  # v augmented with [denominator-ones, pad] columns
CBLOB = 1418  # packed constants blob width (see _pack_consts)

_COMPILED = {}


def _build():
    """Build + compile the single-program SPMD Bass kernel. Cached."""
    if "nc" in _COMPILED:
        return _COMPILED["nc"]

    import concourse.bass as bass
    import concourse.tile as tile
    from concourse import bacc, mybir

    f32 = mybir.dt.float32
    f32r = mybir.dt.float32r
    bf16 = mybir.dt.bfloat16
    AF = mybir.ActivationFunctionType
    OP = mybir.AluOpType

    nc = bacc.Bacc(
        "TRN2",
        target_bir_lowering=False,
        debug=False,
        enable_asserts=True,
        num_devices=N_CORES,
    )

    # ---- I/O ----
    xT = nc.dram_tensor("xT", [CH, N], f32, kind="ExternalInput").ap()
    xres = nc.dram_tensor("xres", [NQ, CH], f32, kind="ExternalInput").ap()
    cblob_d = nc.dram_tensor("cblob", [128, CBLOB], f32, kind="ExternalInput").ap()
    y = nc.dram_tensor("y", [NQ, CH], f32, kind="ExternalOutput").ap()

    MT = N // 128  # 32 key tiles
    NS = NQ // 512  # 4 query slices
    NG = MT // 4  # 8 groups of 4 key tiles (one scores+exp block each)
    NREG = N // 512  # 8 x-chunk regions

    with tile.TileContext(nc) as tc:
        with (
            tc.tile_pool(name="consts", bufs=1) as consts,
            tc.tile_pool(name="xtp", bufs=1) as xtp,
            tc.tile_pool(name="qk", bufs=1) as qkp,
            tc.tile_pool(name="vp", bufs=1) as vp,
            tc.tile_pool(name="xrp", bufs=1) as xrp,
            tc.tile_pool(name="expp", bufs=10) as expp,
            tc.tile_pool(name="yp", bufs=2) as yp,
            tc.tile_pool(name="smallp", bufs=8) as smallp,
            # scores PSUM: one [128, 4, 512] block = 4 banks, single buffer.
            # It spans the whole kernel (slice-0 scores run during the
            # projection phase).
            tc.tile_pool(name="pss", bufs=1, space="PSUM") as pss,
        ):
            cb = consts.tile([128, CBLOB], f32r)
            # ---- DMA plan: descriptors spread over 4 engine queues so the
            # first bytes land ~2us after kernel start. cbA (wq/wk) gates the
            # warmup; x chunks gate projection regions.
            nc.sync.dma_start(cb[:, 0:512], cblob_d[:, 0:512].bitcast(f32r))
            nc.gpsimd.dma_start(cb[:, 512:CBLOB], cblob_d[:, 512:CBLOB].bitcast(f32r))

            xts = xtp.tile([128, 2, N], f32r)
            xTr = xT.rearrange("(t p) n -> p t n", p=128)
            for t in range(4):
                nc.sync.dma_start(
                    xts[:, :, 512 * t : 512 * (t + 1)],
                    xTr[:, :, 512 * t : 512 * (t + 1)].bitcast(f32r),
                )
            # NOTE: keep the ScalarE queue free of DMA descriptor builds
            # (~800ns each) - it must start the exp stream ASAP.
            for t in range(4, 8):
                nc.gpsimd.dma_start(
                    xts[:, :, 512 * t : 512 * (t + 1)],
                    xTr[:, :, 512 * t : 512 * (t + 1)].bitcast(f32r),
                )
            xr = xrp.tile([128, NQ // 128, CH], f32)
            nc.gpsimd.dma_start(xr[:], xres.rearrange("(t p) c -> p t c", p=128))

            # views into the blob (layout must match _pack_consts)
            wq4s = lambda kt: cb[:, 128 * kt : 128 * (kt + 1)]
            wk4s = lambda kt: cb[:, 256 + 128 * kt : 256 + 128 * (kt + 1)]
            wvs = lambda kt: cb[:, 512 + CH2 * kt : 512 + CH2 * (kt + 1)]
            bq4s = cb[:, 1028:1029].bitcast(f32)
            bk4s = cb[:, 1029:1030].bitcast(f32)
            bvs = cb[0:1, 1030 : 1030 + CH2]
            gs = cb[0:1, 1288:1290]
            oness = cb[0:1, 1290:1418]

            qt4 = qkp.tile([128, NQ], f32r)
            kt4 = qkp.tile([128, N], f32r)
            vaug = vp.tile([128, MT, CH2], bf16)

            def scores_grp(ns, g, s):
                # 4 concurrent K=32 row-strip matmuls (tile_position), one
                # per key tile of the group; each strip contracts one q/k
                # replica.
                for r in range(4):
                    mt = 4 * g + r
                    nc.tensor.matmul(
                        s[:, r, :],
                        lhsT=kt4[32 * r : 32 * (r + 1), 128 * mt : 128 * (mt + 1)],
                        rhs=qt4[32 * r : 32 * (r + 1), 512 * ns : 512 * (ns + 1)],
                        start=True,
                        stop=True,
                        tile_position=(32 * r, 0),
                    )

            e_tiles = {}

            def s_exp(idx):
                ns, g = divmod(idx, NG)
                s = pss.tile([128, 4, 512], f32, tag="s", name=f"s{idx}")
                scores_grp(ns, g, s)
                e = expp.tile([128, 4, 512], bf16, tag="e", name=f"e{idx}")
                nc.scalar.activation(e[:], s[:], AF.Exp)
                e_tiles[idx] = e

            # ---------------- phase A: head + projections ----------------
            with (
                tc.tile_pool(name="psqk", bufs=2, space="PSUM") as psqk,
                tc.tile_pool(name="psv", bufs=1, space="PSUM") as psv,
            ):
                # PE warmup (HAM clock ungate): dummy matmuls gated only on
                # the small cbA DMA; exp-table preload on ACT in parallel.
                warm_sink = consts.tile([128, 1], f32)
                for w in range(5):
                    wt = psqk.tile([128, 512], f32, tag="pqk", name=f"warm{w}")
                    nc.tensor.matmul(
                        wt[:], lhsT=cb[:, 0:128], rhs=cb[:, 0:512],
                        start=True, stop=True,
                    )
                    if w == 4:
                        nc.vector.tensor_reduce(
                            warm_sink[:], wt[:], axis=mybir.AxisListType.X,
                            op=OP.max,
                        )
                warm_exp = consts.tile([1, 2], f32)
                nc.scalar.activation(warm_exp[:], cb[0:1, 0:2].bitcast(f32), AF.Exp)

                # bias/gamma broadcasts via K=1 outer-product matmuls
                pbg = psv.tile([128, 2, 512], f32, tag="pv", name="pbg")
                nc.tensor.matmul(
                    pbg[:, 0, 0:CH2], lhsT=oness.bitcast(f32r),
                    rhs=bvs.bitcast(f32r), start=True, stop=True,
                )
                nc.tensor.matmul(
                    pbg[:, 1, 0:2], lhsT=oness.bitcast(f32r),
                    rhs=gs.bitcast(f32r), start=True, stop=True,
                )
                bvb2 = consts.tile([128, 2, CH2], f32)
                nc.vector.tensor_copy(bvb2[:, 0, :], pbg[:, 0, 0:CH2])
                nc.vector.tensor_copy(bvb2[:, 1, :], pbg[:, 0, 0:CH2])
                gb = consts.tile([128, 2], f32)
                nc.vector.tensor_copy(gb[:], pbg[:, 1, 0:2])

                # projections, interleaved with x-chunk arrival per region;
                # slice-0 scores+exp issued as soon as each key region lands
                # so the ScalarE exp stream starts immediately.
                for t in range(NREG):
                    if t < NS:
                        pq = psqk.tile([128, 512], f32, tag="pqk", name=f"pq{t}")
                        for kt in range(2):
                            nc.tensor.matmul(
                                pq[:],
                                lhsT=wq4s(kt).bitcast(f32r),
                                rhs=xts[:, kt, 512 * t : 512 * (t + 1)].bitcast(f32r),
                                start=(kt == 0),
                                stop=(kt == 1),
                            )
                        nc.vector.tensor_scalar_add(
                            qt4[:, 512 * t : 512 * (t + 1)], pq[:], bq4s
                        )
                    pk = psqk.tile([128, 512], f32, tag="pqk", name=f"pk{t}")
                    for kt in range(2):
                        nc.tensor.matmul(
                            pk[:],
                            lhsT=wk4s(kt).bitcast(f32r),
                            rhs=xts[:, kt, 512 * t : 512 * (t + 1)].bitcast(f32r),
                            start=(kt == 0),
                            stop=(kt == 1),
                        )
                    nc.vector.tensor_scalar_add(
                        kt4[:, 512 * t : 512 * (t + 1)], pk[:], bk4s
                    )
                    # v for key tiles 4t..4t+3, evacuated in pairs
                    for half in range(2):
                        pv = psv.tile([128, 2, 512], f32, tag="pv", name=f"pv{t}_{half}")
                        for i in range(2):
                            mt = 4 * t + 2 * half + i
                            for kt in range(2):
                                nc.tensor.matmul(
                                    pv[:, i, 0:CH2],
                                    lhsT=xts[:, kt, 128 * mt : 128 * (mt + 1)].bitcast(f32r),
                                    rhs=wvs(kt).bitcast(f32r),
                                    start=(kt == 0),
                                    stop=(kt == 1),
                                )
                        mt0 = 4 * t + 2 * half
                        nc.vector.tensor_tensor(
                            vaug[:, mt0 : mt0 + 2, :], pv[:, :, 0:CH2], bvb2[:],
                            op=OP.add,
                        )
                    # slice-0 scores for this key region (q slice 0 is ready
                    # after region 0, key region g == x region g)
                    s_exp(t)

            # ---------------- phase B: attention main loop ----------------
            with tc.tile_pool(name="psa", bufs=1, space="PSUM") as psa:
                acc = None
                for idx in range(NS * NG):
                    ns, g = divmod(idx, NG)
                    # keep one scores+exp block in flight ahead of attn@v
                    if idx + 1 < NS * NG and idx + 1 >= NG:
                        s_exp(idx + 1)
                    if g == 0:
                        acc = psa.tile([128, 4, 512], f32, tag="acc", name=f"acc{ns}")
                    e = e_tiles.pop(idx)
                    # acc[n, :] += expT[m, n].T-as-weights @ v_aug[m, :]
                    for r in range(4):
                        mt = 4 * g + r
                        for j in range(4):
                            nc.tensor.matmul(
                                acc[:, j, 0:CH2],
                                lhsT=e[:, r, 128 * j : 128 * (j + 1)],
                                rhs=vaug[:, mt, :],
                                start=(g == 0 and r == 0),
                                stop=(g == NG - 1 and r == 3),
                            )
                    if g == NG - 1:
                        # evacuate acc quickly (one copy) so the next slice's
                        # accumulation isn't blocked on the normalize chain
                        accs = yp.tile([128, 4, CH2], f32, tag="accs")
                        nc.vector.tensor_copy(accs[:], acc[:, :, 0:CH2])
                        yt = yp.tile([128, 4, CH], f32, tag="yt")
                        for j in range(4):
                            nt = 4 * ns + j
                            r_ = smallp.tile([128, 1], f32)
                            nc.vector.reciprocal(r_[:], accs[:, j, CH : CH + 1])
                            rg = smallp.tile([128, 1], f32)
                            nc.vector.tensor_tensor(
                                rg[:], r_[:], gb[:, 0:1], op=OP.mult
                            )
                            nc.vector.scalar_tensor_tensor(
                                yt[:, j, :],
                                accs[:, j, 0:CH],
                                rg[:, 0:1],
                                xr[:, nt, :],
                                op0=OP.mult,
                                op1=OP.add,
                            )
                        nc.gpsimd.dma_start(
                            y.rearrange("(t p) c -> p t c", p=128)[
                                :, 4 * ns : 4 * (ns + 1), :
                            ],
                            yt[:],
                        )

    nc.compile()
    _COMPILED["nc"] = nc
    return nc


def _pack_consts(Wq, bq, Wk, bk, Wv, bv, gamma):
    """Pack all small constants into one [128, CBLOB] blob.

    Layout (per partition p):
      [0:256)     Wq4 k-tiles: [wq4[p], wq4[p+128]]   (wq4 = tile(Wq, (1,4)))
      [256:512)   Wk4 k-tiles
      [512:1028)  Wv_aug k-tiles (CH2 = 258 each)
      [1028]      bq4[p];  [1029] bk4[p]
      partition 0 only:
      [1030:1288) bv_aug (bv ++ [1.0, 0.0])
      [1288:1290) gamma, 0
      [1290:1418) ones
    """
    Wq4 = np.tile(np.asarray(Wq, np.float32), (1, 4))  # [256, 128]
    Wk4 = np.tile(np.asarray(Wk, np.float32), (1, 4))
    bq4 = np.tile(np.asarray(bq, np.float32), 4)  # [128]
    bk4 = np.tile(np.asarray(bk, np.float32), 4)
    Wv_aug = np.zeros((CH, CH2), np.float32)
    Wv_aug[:, :CH] = np.asarray(Wv, np.float32)

    cb = np.zeros((128, CBLOB), np.float32)
    for kt in range(2):
        cb[:, 128 * kt : 128 * (kt + 1)] = Wq4[128 * kt : 128 * (kt + 1), :]
        cb[:, 256 + 128 * kt : 256 + 128 * (kt + 1)] = Wk4[128 * kt : 128 * (kt + 1)]
        cb[:, 512 + CH2 * kt : 512 + CH2 * (kt + 1)] = Wv_aug[
            128 * kt : 128 * (kt + 1), :
        ]
    cb[:, 1028] = bq4
    cb[:, 1029] = bk4
    cb[0, 1030 : 1030 + CH] = np.asarray(bv, np.float32)
    cb[0, 1030 + CH] = 1.0
    cb[0, 1288] = np.float32(np.asarray(gamma).reshape(()))
    cb[0, 1290:1418] = 1.0
    return cb


def _shard_inputs(x, Wq, bq, Wk, bk, Wv, bv, gamma):
    """Host-side prep: one input map per core."""
    xf = np.ascontiguousarray(x, dtype=np.float32).reshape(B, N, CH)
    cb = _pack_consts(Wq, bq, Wk, bk, Wv, bv, gamma)

    in_maps = []
    for c in range(N_CORES):
        b, h = divmod(c, 2)
        own = slice(h * NQ, (h + 1) * NQ)
        other = slice((1 - h) * NQ, (2 - h) * NQ)
        xT_b = xf[b].T  # [CH, N]
        xT_roll = np.ascontiguousarray(
            np.concatenate([xT_b[:, own], xT_b[:, other]], axis=1)
        )
        in_maps.append(
            {
                "xT": xT_roll,
                "xres": np.ascontiguousarray(xf[b, own]),
                "cblob": cb,
            }
        )
    return in_maps


def kernel(x, Wq, bq, Wk, bk, Wv, bv, gamma):
    from concourse.bass_utils import run_bass_kernel_spmd

    nc = _build()
    in_maps = _shard_inputs(x, Wq, bq, Wk, bk, Wv, bv, gamma)
    res = run_bass_kernel_spmd(nc, in_maps, core_ids=list(range(N_CORES)))
    out = np.empty((B, N, CH), np.float32)
    for c in range(N_CORES):
        b, h = divmod(c, 2)
        out[b, h * NQ : (h + 1) * NQ, :] = res.results[c]["y"]
    return out.reshape(x.shape)
